# revision 1
# baseline (speedup 1.0000x reference)
"""Trainium2 raw-Bass kernel for a 5-layer MLP over graph nodes (ChebConv K=1).

Network: x[50000,512] -> ELU(x@W1+b1) -> ... -> h@W5+b5, dims 512->2048(x4)->256.
ChebConv(K=1) + parallel Linear fuse on the host: W = Wg+Wl, b = bg+bl.
edge_index is unused (no neighbor exchange for K=1).

Sharding: data-parallel over nodes, 6250 nodes/core on 8 NeuronCores, weights
replicated, no collectives.

Raw Bass (manual semaphores, no Tile framework):
  - activations feature-major in SBUF as [128 feat-part, kblk, nodes] bf16;
    x host-pre-transposed/cast (no device transpose).
  - node blocks 1536/1536/1536/1642; mid layers run m(16) x ch x k(kc) matmul
    groups into 6 rotating PSUM slots, one per bank (bank-exclusive => no
    PE-W/DVE-R same-bank hazards; the s_ev>=t-5 wait exactly covers reuse).
  - L1 runs ch-major with the whole W1 resident in SBUF (one const DMA), so
    its evictions complete chunk-by-chunk and L2 needs only a per-chunk
    barrier instead of a full-layer one (PE never idles at the L1->L2 edge).
  - ELU eviction per tile: r=relu(z+b) [ACT], mn=min(z+b,0) [DVE],
    e=exp(mn) [ACT], h=(r-1)+e [DVE scalar_tensor_tensor]; temps in bf16.
  - layer 5 flips the mapping (lhsT = activation chunk, moving = W5) to give
    node-major [<=128, 256] PSUM tiles on banks 6/7, ACT-copied and DMA'd out.
  - W2..W4 stream from DRAM through 6 rotating SBUF buffers, re-fetched per
    node block (4 x 24MB per pass, ~12% DMA duty, hidden).
  - `passes` repeats the whole computation inside one NEFF; test.py times
    T(passes=hi) - T(passes=lo) to cancel the noisy axon dispatch cost.
"""

import numpy as np
import ml_dtypes

N = 50000
IN_C = 512
DIM = 2048
OUT_C = 256
NCORES = 8
NPC = N // NCORES  # 6250
BLOCKS = [1536, 1536, 1536, 1642]
assert sum(BLOCKS) == NPC
NBMAX = max(BLOCKS)
LAYER_DIMS = [(IN_C, DIM), (DIM, DIM), (DIM, DIM), (DIM, DIM), (DIM, OUT_C)]
NWBUF = 6  # rotating weight buffers (layers 2..4)
NMID_PS = 6  # PSUM slots for mid layers (one per bank 0..5)

_cache = {}


def _chunks(total, step):
    out = []
    o = 0
    while o < total:
        c = min(step, total - o)
        out.append((o, c))
        o += c
    return out


def _build(n_mm=512, passes=1):
    from contextlib import ExitStack

    from concourse import bacc, mybir

    f32 = mybir.dt.float32
    bf16 = mybir.dt.bfloat16
    AF = mybir.ActivationFunctionType
    ALU = mybir.AluOpType

    nc = bacc.Bacc(
        trn_type="TRN2", target_bir_lowering=False, debug=False, num_devices=NCORES
    )

    x_h = nc.dram_tensor("x", [128, IN_C // 128, NPC], bf16, kind="ExternalInput")
    # W1 packed [128 part, 16 m, 4 k, 128] - fully resident in SBUF
    w1_h = nc.dram_tensor("w1", [128, 16, 4, 128], bf16, kind="ExternalInput")
    w_h = {}
    for l in (2, 3, 4):
        w_h[l] = nc.dram_tensor(
            f"w{l}", [DIM // 128, 128, 16, 128], bf16, kind="ExternalInput"
        )
    w5_h = nc.dram_tensor("w5", [128, DIM // 128, OUT_C], bf16, kind="ExternalInput")
    b_h = nc.dram_tensor("bmin", [128, 5, 16], f32, kind="ExternalInput")
    out_h = nc.dram_tensor("out", [NPC, OUT_C], f32, kind="ExternalOutput")

    x_ap = x_h.ap()
    out_ap = out_h.ap()

    NBLK = len(BLOCKS)
    block_n0 = np.cumsum([0] + BLOCKS).tolist()
    # streamed weight tiles per block: layers 2..4
    WT_PER_BLOCK = [(l, m) for l in (2, 3, 4) for m in range(16)]
    L5_CHUNKS = [_chunks(BLOCKS[b], 128) for b in range(NBLK)]

    # mid-tile emission order per block: L1 ch-major, L2-4 m-major
    def tiles_of_block(b):
        nb = BLOCKS[b]
        ch_list = _chunks(nb, n_mm)
        tiles = []
        for ci, (off, cs) in enumerate(ch_list):  # L1: ch-major
            for m in range(16):
                tiles.append((0, m, ci, off, cs))
        for li in (1, 2, 3):  # L2-4: m-major
            for m in range(16):
                for ci, (off, cs) in enumerate(ch_list):
                    tiles.append((li, m, ci, off, cs))
        return tiles

    TILES = [tiles_of_block(b) for b in range(NBLK)]
    CH = [len(_chunks(BLOCKS[b], n_mm)) for b in range(NBLK)]

    # cumulative mid-group count after each streamed weight tile / each L1
    wt_release = []
    x_release = []
    _mc = 0
    for _p in range(passes):
        for _b in range(NBLK):
            _mc += 16 * CH[_b]  # L1 tiles
            x_release.append(_mc)
            for _l in (1, 2, 3):
                for _m in range(16):
                    _mc += CH[_b]
                    wt_release.append(_mc)

    with ExitStack() as ctx:
        xf = ctx.enter_context(nc.sbuf_tensor("xf", [128, 2, IN_C // 128, NBMAX], bf16))
        hb = ctx.enter_context(nc.sbuf_tensor("hb", [128, 2, DIM // 128, NBMAX], bf16))
        w1sb = ctx.enter_context(nc.sbuf_tensor("w1sb", [128, 16, 4, 128], bf16))
        wb = ctx.enter_context(nc.sbuf_tensor("wb", [128, NWBUF, 16, 128], bf16))
        w5sb = ctx.enter_context(nc.sbuf_tensor("w5sb", [128, DIM // 128, OUT_C], bf16))
        bsb = ctx.enter_context(nc.sbuf_tensor("bsb", [128, 5, 16], f32))
        rt = ctx.enter_context(nc.sbuf_tensor("rt", [128, 2, 512], bf16))
        mnt = ctx.enter_context(nc.sbuf_tensor("mnt", [128, 4, 512], bf16))
        ext = ctx.enter_context(nc.sbuf_tensor("ext", [128, 2, 512], bf16))
        osb = ctx.enter_context(nc.sbuf_tensor("osb", [128, 4, OUT_C], f32))
        pz = ctx.enter_context(nc.psum_tensor("pz", [128, NMID_PS, 512], f32))
        p5 = ctx.enter_context(nc.psum_tensor("p5", [128, 2, 512], f32))
        s_cw = ctx.enter_context(nc.semaphore("s_cw"))  # const DMAs (16 each)
        s_xs = [ctx.enter_context(nc.semaphore(f"s_x{i}")) for i in range(2)]
        s_wb = [ctx.enter_context(nc.semaphore(f"s_wb{i}")) for i in range(NWBUF)]
        s_pm = ctx.enter_context(nc.semaphore("s_pm"))  # PE mid group complete
        s_mn = ctx.enter_context(nc.semaphore("s_mn"))  # DVE min done
        s_ex = ctx.enter_context(nc.semaphore("s_ex"))  # ACT exp done
        s_ev = ctx.enter_context(nc.semaphore("s_ev"))  # DVE stt done
        s_p5 = ctx.enter_context(nc.semaphore("s_p5"))  # PE L5 chunk complete
        s_oc = ctx.enter_context(nc.semaphore("s_oc"))  # ACT out-copy done
        s_ods = [ctx.enter_context(nc.semaphore(f"s_od{i}")) for i in range(4)]
        block = ctx.enter_context(nc.Block())
        zero_ap = bsb[:, 4, 0:1]

        @block.sync
        def _(sync):
            wt = 0
            oj = 0
            for p in range(passes):
                for b in range(NBLK):
                    g = p * NBLK + b
                    n0 = block_n0[b]
                    if g >= 2:
                        sync.wait_ge(s_pm, x_release[g - 2])
                    sync.dma_start(
                        xf[:, g % 2, :, : BLOCKS[b]], x_ap[:, :, n0 : n0 + BLOCKS[b]]
                    ).then_inc(s_xs[g % 2], 16)
                    if g == 0:
                        sync.dma_start(bsb[:], b_h.ap()).then_inc(s_cw, 16)
                        sync.dma_start(w1sb[:], w1_h.ap()).then_inc(s_cw, 16)
                        sync.dma_start(w5sb[:], w5_h.ap()).then_inc(s_cw, 16)
                    # first NWBUF streamed tiles of block g (their back-pressure
                    # resolves during block g-1 -> no head-of-line blocking)
                    for l, m in WT_PER_BLOCK[:NWBUF]:
                        if wt >= NWBUF:
                            sync.wait_ge(s_pm, wt_release[wt - NWBUF])
                        sync.dma_start(
                            wb[:, wt % NWBUF, :, :], w_h[l].ap()[m]
                        ).then_inc(s_wb[wt % NWBUF], 16)
                        wt += 1
                    # out DMAs of previous block (b==0's predecessor was
                    # issued by the previous pass's tail)
                    if b >= 1:
                        n0p = block_n0[b - 1]
                        for c0, csz in L5_CHUNKS[b - 1]:
                            sync.wait_ge(s_oc, oj + 1)
                            sync.dma_start(
                                out_ap[n0p + c0 : n0p + c0 + csz, :],
                                osb[:csz, oj % 4, :],
                            ).then_inc(s_ods[oj % 4], 16)
                            oj += 1
                    # remaining streamed tiles for block g
                    for l, m in WT_PER_BLOCK[NWBUF:]:
                        sync.wait_ge(s_pm, wt_release[wt - NWBUF])
                        sync.dma_start(
                            wb[:, wt % NWBUF, :, :], w_h[l].ap()[m]
                        ).then_inc(s_wb[wt % NWBUF], 16)
                        wt += 1
                # tail: final block's out DMAs
                n0p = block_n0[NBLK - 1]
                for c0, csz in L5_CHUNKS[NBLK - 1]:
                    sync.wait_ge(s_oc, oj + 1)
                    sync.dma_start(
                        out_ap[n0p + c0 : n0p + c0 + csz, :], osb[:csz, oj % 4, :]
                    ).then_inc(s_ods[oj % 4], 16)
                    oj += 1
            for i in range(4):
                cnt = oj // 4 + (1 if oj % 4 > i else 0)
                if cnt:
                    sync.wait_ge(s_ods[i], 16 * cnt)

        @block.tensor
        def _(tensor):
            tensor.wait_ge(s_cw, 48)
            wt = 0
            mt = 0
            jc = 0
            for p in range(passes):
                for b in range(NBLK):
                    g = p * NBLK + b
                    nb = BLOCKS[b]
                    ch_list = _chunks(nb, n_mm)
                    tensor.wait_ge(s_xs[g % 2], 16 * (g // 2 + 1))
                    l1_base = mt
                    # ---- L1: ch-major, resident W1
                    for ci, (off, cs) in enumerate(ch_list):
                        for m in range(16):
                            if mt >= NMID_PS:
                                tensor.wait_ge(s_ev, mt - (NMID_PS - 1))
                            slot = mt % NMID_PS
                            last = None
                            for k in range(4):
                                last = tensor.matmul(
                                    pz[:, slot, :cs],
                                    w1sb[:, m, k, :],
                                    xf[:, g % 2, k, off : off + cs],
                                    start=(k == 0),
                                    stop=(k == 3),
                                )
                            last.then_inc(s_pm, 1)
                            mt += 1
                    # ---- L2-4: m-major, streamed weights
                    for li in (1, 2, 3):
                        layer_start = mt
                        hi = (li - 1) % 2
                        for m in range(16):
                            tensor.wait_ge(
                                s_wb[wt % NWBUF], 16 * (wt // NWBUF + 1)
                            )
                            for ci, (off, cs) in enumerate(ch_list):
                                if li == 1:
                                    # per-chunk barrier on L1's ch-major evictions
                                    tensor.wait_ge(s_ev, l1_base + 16 * (ci + 1))
                                elif m == 0 and ci == 0:
                                    # full barrier on previous layer
                                    tensor.wait_ge(s_ev, layer_start)
                                if mt >= NMID_PS:
                                    tensor.wait_ge(s_ev, mt - (NMID_PS - 1))
                                slot = mt % NMID_PS
                                last = None
                                for k in range(16):
                                    last = tensor.matmul(
                                        pz[:, slot, :cs],
                                        wb[:, wt % NWBUF, k, :],
                                        hb[:, hi, k, off : off + cs],
                                        start=(k == 0),
                                        stop=(k == 15),
                                    )
                                last.then_inc(s_pm, 1)
                                mt += 1
                            wt += 1
                    # ---- L5
                    tensor.wait_ge(s_ev, mt)
                    for c0, csz in L5_CHUNKS[b]:
                        if jc >= 2:
                            tensor.wait_ge(s_oc, jc - 1)
                        last = None
                        for k in range(DIM // 128):
                            last = tensor.matmul(
                                p5[:csz, jc % 2, :OUT_C],
                                hb[:, 1, k, c0 : c0 + csz],
                                w5sb[:, k, :],
                                start=(k == 0),
                                stop=(k == DIM // 128 - 1),
                            )
                        last.then_inc(s_p5, 1)
                        jc += 1

        @block.scalar
        def _(scalar):
            scalar.wait_ge(s_cw, 48)
            mt = 0
            jc = 0
            for p in range(passes):
                for b in range(NBLK):
                    for li, m, ci, off, cs in TILES[b]:
                        bias = bsb[:, li, m : m + 1]
                        if mt >= 2:
                            scalar.wait_ge(s_ev, mt - 1)
                        scalar.wait_ge(s_mn, mt + 1)
                        scalar.activation(
                            rt[:, mt % 2, :cs],
                            pz[:, mt % NMID_PS, :cs],
                            AF.Relu,
                            bias=bias,
                            scale=1.0,
                        )
                        scalar.activation(
                            ext[:, mt % 2, :cs],
                            mnt[:, mt % 4, :cs],
                            AF.Exp,
                            bias=zero_ap,
                            scale=1.0,
                        ).then_inc(s_ex, 1)
                        mt += 1
                    # L5 psum -> sbuf copies
                    for c0, csz in L5_CHUNKS[b]:
                        scalar.wait_ge(s_p5, jc + 1)
                        if jc >= 4:
                            scalar.wait_ge(s_ods[jc % 4], 16 * ((jc - 4) // 4 + 1))
                        scalar.copy(
                            osb[:csz, jc % 4, :], p5[:csz, jc % 2, :OUT_C]
                        ).then_inc(s_oc, 1)
                        jc += 1

        @block.vector
        def _(vector):
            vector.wait_ge(s_cw, 48)
            mt = 0

            def emit_stt(cs, out_ap_, t):
                vector.wait_ge(s_ex, t + 1)
                vector.scalar_tensor_tensor(
                    out_ap_,
                    rt[:, t % 2, :cs],
                    -1.0,
                    ext[:, t % 2, :cs],
                    ALU.add,
                    ALU.add,
                ).then_inc(s_ev, 1)

            for p in range(passes):
                for b in range(NBLK):
                    # split the block's tiles by layer (PE barriers are per
                    # layer, so pending stt's flush at each layer end)
                    by_layer = {}
                    for tile in TILES[b]:
                        by_layer.setdefault(tile[0], []).append(tile)
                    for li in sorted(by_layer):
                        sub = by_layer[li]
                        pend = []
                        for (li_, m, ci, off, cs) in sub:
                            bias = bsb[:, li_, m : m + 1]
                            ho = li_ % 2 if li_ > 0 else 0
                            vector.wait_ge(s_pm, mt + 1)
                            vector.tensor_scalar(
                                mnt[:, mt % 4, :cs],
                                pz[:, mt % NMID_PS, :cs],
                                bias,
                                0.0,
                                ALU.add,
                                ALU.min,
                            ).then_inc(s_mn, 1)
                            pend.append((cs, hb[:, ho, m, off : off + cs], mt))
                            mt += 1
                            if len(pend) > 2:
                                emit_stt(*pend.pop(0))
                        for e in pend:
                            emit_stt(*e)

    nc.compile()
    return nc


def _prep_weights(inputs):
    bf16 = ml_dtypes.bfloat16
    wmaps = {}
    bmin = np.zeros((128, 5, 16), np.float32)
    for l, (ci, co) in enumerate(LAYER_DIMS, start=1):
        W = np.asarray(inputs[f"Wg{l}"], np.float32) + np.asarray(
            inputs[f"Wl{l}"], np.float32
        )
        b = np.asarray(inputs[f"bg{l}"], np.float32) + np.asarray(
            inputs[f"bl{l}"], np.float32
        )
        kc = ci // 128
        if l == 1:
            # [128 part, 16 m, 4 k, 128]
            wmaps["w1"] = np.ascontiguousarray(
                W.reshape(4, 128, 16, 128).transpose(1, 2, 0, 3)
            ).astype(bf16)
            bmin[:, 0, :] = b.reshape(16, 128).T
        elif l < 5:
            wmaps[f"w{l}"] = np.ascontiguousarray(
                W.reshape(kc, 128, co // 128, 128).transpose(2, 1, 0, 3)
            ).astype(bf16)
            bmin[:, l - 1, :] = b.reshape(co // 128, 128).T
        else:
            wmaps["w5"] = np.ascontiguousarray(
                W.reshape(ci // 128, 128, co).transpose(1, 0, 2)
            ).astype(bf16)
            wmaps["_b5"] = b
    wmaps["bmin"] = bmin
    return wmaps


def make_in_maps(inputs):
    x = np.asarray(inputs["x"], np.float32)
    assert x.shape == (N, IN_C)
    wmaps = _prep_weights(inputs)
    b5 = wmaps.pop("_b5")
    in_maps = []
    for c in range(NCORES):
        xs = x[c * NPC : (c + 1) * NPC]
        xt = np.ascontiguousarray(
            xs.T.reshape(IN_C // 128, 128, NPC).transpose(1, 0, 2)
        ).astype(ml_dtypes.bfloat16)
        m = {"x": xt}
        m.update(wmaps)
        in_maps.append(m)
    return in_maps, b5


def kernel(**inputs) -> np.ndarray:
    from concourse.bass_utils import run_bass_kernel_spmd

    in_maps, b5 = make_in_maps(inputs)

    if "nc" not in _cache:
        _cache["nc"] = _build()
    nc = _cache["nc"]

    res = run_bass_kernel_spmd(nc, in_maps, core_ids=list(range(NCORES)))
    out = np.concatenate([res.results[c]["out"] for c in range(NCORES)], axis=0)
    if np.any(b5):
        out = out + b5[None, :]
    return np.ascontiguousarray(out.astype(np.float32))



# revision 2
# speedup vs baseline: 1.0072x; 1.0072x over previous
"""Trainium2 raw-Bass kernel for a 5-layer MLP over graph nodes (ChebConv K=1).

Network: x[50000,512] -> ELU(x@W1+b1) -> ... -> h@W5+b5, dims 512->2048(x4)->256.
ChebConv(K=1) + parallel Linear fuse on the host: W = Wg+Wl, b = bg+bl.
edge_index is unused (no neighbor exchange for K=1).

Sharding: data-parallel over nodes, 6250 nodes/core on 8 NeuronCores, weights
replicated, no collectives.

v2 design (raw Bass, manual semaphores):
  - activations feature-major in SBUF as [128 feat-part, kblk, nodes] bf16;
    x host-pre-transposed/cast (no device transpose).
  - node blocks 1536/1536/1536/1642, chunks of 512 (+ ragged 106); tiles
    (m, ci) run a k-inner accumulation into 6 rotating PSUM slots.
  - 3-op ELU eviction per tile, ELU(v) = max(v, min(exp(v)-1, 0)):
      ACT: e = exp(pz + b)        (bias folded into the activation)
      DVE: t = min(e - 1, 0)
      DVE: hb = max(pz + b, t)    (scalar_tensor_tensor, AP bias)
  - NO layer barriers: layer li+1's k-th accumulation step only needs the
    previous layer's m=k tile evicted, so the first tiles of each layer use
    a per-k "ladder" of s_ev waits (prev_base + k*ch + ci + 1). The PE never
    drains at layer boundaries; evictions of the tail tiles overlap the next
    layer's early k steps.
  - same ladder lets L1 start on the first x k-piece (x DMA'd in 4 per-k
    pieces per block) and W1 m-piece at startup.
  - L5 flips the mapping (lhsT = activation chunk, moving = W5) giving
    node-major [<=128, 256] PSUM tiles on banks 6/7, ACT-copied, DMA'd out.
  - W2..W4 stream from DRAM through 6 rotating SBUF buffers, re-fetched per
    node block (4 x 24MB per pass, hidden under compute).
  - `passes` repeats the computation inside one NEFF for marginal-cost
    timing (passes-delta cancels axon dispatch overhead).
"""

import numpy as np
import ml_dtypes

N = 50000
IN_C = 512
DIM = 2048
OUT_C = 256
NCORES = 8
NPC = N // NCORES  # 6250
BLOCKS = [1536, 1536, 1536, 1642]
assert sum(BLOCKS) == NPC
NBMAX = max(BLOCKS)
LAYER_DIMS = [(IN_C, DIM), (DIM, DIM), (DIM, DIM), (DIM, DIM), (DIM, OUT_C)]
NWBUF = 6  # rotating weight buffers (layers 2..4)
NPS = 6  # PSUM slots for L1-L4 tiles (banks 0..5)
EXTD = 6  # exp temp depth (>=5 so s_ev>=t-5 guards reuse transitively)
TMND = 2

_cache = {}


def _chunks(total, step):
    out = []
    o = 0
    while o < total:
        c = min(step, total - o)
        out.append((o, c))
        o += c
    return out


def _build(passes=1):
    from contextlib import ExitStack

    from concourse import bacc, mybir

    f32 = mybir.dt.float32
    bf16 = mybir.dt.bfloat16
    AF = mybir.ActivationFunctionType
    ALU = mybir.AluOpType

    nc = bacc.Bacc(
        trn_type="TRN2", target_bir_lowering=False, debug=False, num_devices=NCORES
    )

    x_h = nc.dram_tensor("x", [128, IN_C // 128, NPC], bf16, kind="ExternalInput")
    w1_h = nc.dram_tensor("w1", [128, 16, 4, 128], bf16, kind="ExternalInput")
    w_h = {}
    for l in (2, 3, 4):
        w_h[l] = nc.dram_tensor(
            f"w{l}", [DIM // 128, 128, 16, 128], bf16, kind="ExternalInput"
        )
    w5_h = nc.dram_tensor("w5", [128, DIM // 128, OUT_C], bf16, kind="ExternalInput")
    b_h = nc.dram_tensor("bmin", [128, 5, 16], f32, kind="ExternalInput")
    out_h = nc.dram_tensor("out", [NPC, OUT_C], f32, kind="ExternalOutput")

    x_ap = x_h.ap()
    out_ap = out_h.ap()

    NBLK = len(BLOCKS)
    block_n0 = np.cumsum([0] + BLOCKS).tolist()
    CHUNKS = [_chunks(BLOCKS[b], 512) for b in range(NBLK)]
    CH = [len(c) for c in CHUNKS]
    L5_CHUNKS = [_chunks(BLOCKS[b], 128) for b in range(NBLK)]

    # ---- global tile schedule (shared by PE / ACT / DVE blocks) ----
    # tile = (li, m, ci, off, cs); per block: L1..L4, m-major, ci-inner.
    def tiles_of_block(b):
        tiles = []
        for li in range(4):
            for m in range(16):
                for ci, (off, cs) in enumerate(CHUNKS[b]):
                    tiles.append((li, m, ci, off, cs))
        return tiles

    TILES = [tiles_of_block(b) for b in range(NBLK)]
    # tile-index base of layer li within block b (relative to block start)
    layer_base = [[4 * 16 * CH[b] * 0 + li * 16 * CH[b] for li in range(4)] for b in range(NBLK)]
    block_tiles = [4 * 16 * CH[b] for b in range(NBLK)]
    pass_tiles = sum(block_tiles)

    # streamed weight tile release counts (tile units of s_pm), emission order
    wt_release = []
    for p in range(passes):
        tbase = p * pass_tiles
        for b in range(NBLK):
            for li in (1, 2, 3):
                for m in range(16):
                    # consumed once its ch tiles are done
                    rel = tbase + layer_base[b][li] + (m + 1) * CH[b]
                    wt_release.append(rel)
            tbase += block_tiles[b]

    # L1-end (x release) per block g: s_pm count after L1(g) tiles all done
    x_release = []
    tbase = 0
    for p in range(passes):
        for b in range(NBLK):
            x_release.append(tbase + layer_base[b][1])  # == L1 end
            tbase += block_tiles[b]

    with ExitStack() as ctx:
        xf = ctx.enter_context(nc.sbuf_tensor("xf", [128, 2, IN_C // 128, NBMAX], bf16))
        hb = ctx.enter_context(nc.sbuf_tensor("hb", [128, 2, DIM // 128, NBMAX], bf16))
        w1sb = ctx.enter_context(nc.sbuf_tensor("w1sb", [128, 16, 4, 128], bf16))
        wb = ctx.enter_context(nc.sbuf_tensor("wb", [128, NWBUF, 16, 128], bf16))
        w5sb = ctx.enter_context(nc.sbuf_tensor("w5sb", [128, DIM // 128, OUT_C], bf16))
        bsb = ctx.enter_context(nc.sbuf_tensor("bsb", [128, 5, 16], f32))
        ext = ctx.enter_context(nc.sbuf_tensor("ext", [128, EXTD, 512], bf16))
        tmn = ctx.enter_context(nc.sbuf_tensor("tmn", [128, TMND, 512], bf16))
        osb = ctx.enter_context(nc.sbuf_tensor("osb", [128, 4, OUT_C], f32))
        pz = ctx.enter_context(nc.psum_tensor("pz", [128, NPS, 512], f32))
        p5 = ctx.enter_context(nc.psum_tensor("p5", [128, 2, 512], f32))
        s_cw = ctx.enter_context(nc.semaphore("s_cw"))  # bias DMA
        s_w1 = ctx.enter_context(nc.semaphore("s_w1"))  # w1 m-pieces
        s_w5 = ctx.enter_context(nc.semaphore("s_w5"))  # w5
        s_xs = [ctx.enter_context(nc.semaphore(f"s_x{i}")) for i in range(2)]
        s_wb = [ctx.enter_context(nc.semaphore(f"s_wb{i}")) for i in range(NWBUF)]
        s_pm = ctx.enter_context(nc.semaphore("s_pm"))  # PE tile complete
        s_ex = ctx.enter_context(nc.semaphore("s_ex"))  # ACT exp done
        s_ev = ctx.enter_context(nc.semaphore("s_ev"))  # DVE stt done (tile evicted)
        s_p5 = ctx.enter_context(nc.semaphore("s_p5"))  # PE L5 chunk complete
        s_oc = ctx.enter_context(nc.semaphore("s_oc"))  # ACT out-copy done
        s_ods = [ctx.enter_context(nc.semaphore(f"s_od{i}")) for i in range(4)]
        block = ctx.enter_context(nc.Block())

        @block.sync
        def _(sync):
            wt = 0
            oj = 0
            for p in range(passes):
                for b in range(NBLK):
                    g = p * NBLK + b
                    n0 = block_n0[b]
                    nb = BLOCKS[b]
                    if g == 0:
                        # startup: bias first (tiny), then first x k-piece +
                        # first w1 m-piece so L1 (m0,c0,k0) starts asap.
                        sync.dma_start(bsb[:], b_h.ap()).then_inc(s_cw, 16)
                        sync.dma_start(
                            w1sb[:, 0:1, :, :], w1_h.ap()[:, 0:1, :, :]
                        ).then_inc(s_w1, 16)
                        for k in range(4):
                            sync.dma_start(
                                xf[:, 0, k : k + 1, :nb],
                                x_ap[:, k : k + 1, n0 : n0 + nb],
                            ).then_inc(s_xs[0], 16)
                        for m in range(1, 16):
                            sync.dma_start(
                                w1sb[:, m : m + 1, :, :], w1_h.ap()[:, m : m + 1, :, :]
                            ).then_inc(s_w1, 16)
                        sync.dma_start(w5sb[:], w5_h.ap()).then_inc(s_w5, 16)
                    else:
                        # x for block g arrives during block g-1 (see below)
                        pass
                    # prefetch x for block g+1 into xf[(g+1)%2]
                    if g + 1 < passes * NBLK:
                        g2 = g + 1
                        b2 = g2 % NBLK
                        n02 = block_n0[b2]
                        nb2 = BLOCKS[b2]
                        if g2 >= 2:
                            sync.wait_ge(s_pm, x_release[g2 - 2])
                        for k in range(4):
                            sync.dma_start(
                                xf[:, g2 % 2, k : k + 1, :nb2],
                                x_ap[:, k : k + 1, n02 : n02 + nb2],
                            ).then_inc(s_xs[g2 % 2], 16)
                    # streamed weight tiles for block g (L2-4, 48 tiles)
                    wt_block_start = wt
                    for i in range(48):
                        if wt >= NWBUF:
                            sync.wait_ge(s_pm, wt_release[wt - NWBUF])
                        l = (2, 3, 4)[i // 16]
                        m = i % 16
                        sync.dma_start(
                            wb[:, wt % NWBUF, :, :], w_h[l].ap()[m]
                        ).then_inc(s_wb[wt % NWBUF], 16)
                        wt += 1
                        # after the first NWBUF tiles, drain previous block's
                        # out DMAs (they're ready by now; avoids HOL blocking)
                        if i == NWBUF - 1 and g >= 1:
                            bprev = (g - 1) % NBLK
                            n0p = block_n0[bprev]
                            for c0, csz in L5_CHUNKS[bprev]:
                                sync.wait_ge(s_oc, oj + 1)
                                sync.dma_start(
                                    out_ap[n0p + c0 : n0p + c0 + csz, :],
                                    osb[:csz, oj % 4, :],
                                ).then_inc(s_ods[oj % 4], 16)
                                oj += 1
                # tail of pass: nothing extra (last block's outs drain in the
                # next pass's first block, or after the loop)
            bprev = (passes * NBLK - 1) % NBLK
            n0p = block_n0[bprev]
            for c0, csz in L5_CHUNKS[bprev]:
                sync.wait_ge(s_oc, oj + 1)
                sync.dma_start(
                    out_ap[n0p + c0 : n0p + c0 + csz, :], osb[:csz, oj % 4, :]
                ).then_inc(s_ods[oj % 4], 16)
                oj += 1
            for i in range(4):
                cnt = oj // 4 + (1 if oj % 4 > i else 0)
                if cnt:
                    sync.wait_ge(s_ods[i], 16 * cnt)

        @block.tensor
        def _(tensor):
            ev_hi = [0]  # max s_ev bound waited so far

            def wait_ev(bound):
                if bound > ev_hi[0]:
                    tensor.wait_ge(s_ev, bound)
                    ev_hi[0] = bound

            mt = 0
            wt = 0
            jc = 0
            for p in range(passes):
                for b in range(NBLK):
                    g = p * NBLK + b
                    ch = CH[b]
                    btile = mt  # block tile base (global)
                    # x pieces counter base for this block's buffer
                    xcnt = 64 * (g // 2)
                    # ---- L1 ----
                    for m in range(16):
                        if g == 0:
                            tensor.wait_ge(s_w1, 16 * (m + 1))
                        for ci, (off, cs) in enumerate(CHUNKS[b]):
                            t = mt
                            wait_ev(t - (NPS - 1))
                            last = None
                            for k in range(4):
                                if m == 0 and ci == 0:
                                    tensor.wait_ge(s_xs[g % 2], xcnt + 16 * (k + 1))
                                last = tensor.matmul(
                                    pz[:, t % NPS, :cs],
                                    w1sb[:, m, k, :],
                                    xf[:, g % 2, k, off : off + cs],
                                    start=(k == 0),
                                    stop=(k == 3),
                                )
                            last.then_inc(s_pm, 1)
                            mt += 1
                    # ---- L2-4 ----
                    for li in (1, 2, 3):
                        prev_base = btile + layer_base[b][li - 1]
                        hi = (li - 1) % 2
                        for m in range(16):
                            tensor.wait_ge(s_wb[wt % NWBUF], 16 * (wt // NWBUF + 1))
                            for ci, (off, cs) in enumerate(CHUNKS[b]):
                                t = mt
                                wait_ev(t - (NPS - 1))
                                last = None
                                for k in range(16):
                                    if m == 0 and ci == 0:
                                        wait_ev(prev_base + k * ch + 1)
                                    elif m == 0 and k == 15:
                                        wait_ev(prev_base + 15 * ch + ci + 1)
                                    last = tensor.matmul(
                                        pz[:, t % NPS, :cs],
                                        wb[:, wt % NWBUF, k, :],
                                        hb[:, hi, k, off : off + cs],
                                        start=(k == 0),
                                        stop=(k == 15),
                                    )
                                last.then_inc(s_pm, 1)
                                mt += 1
                            wt += 1
                    # ---- L5 ----
                    if g == 0:
                        tensor.wait_ge(s_w5, 16)
                    prev_base = btile + layer_base[b][3]
                    for c0, csz in L5_CHUNKS[b]:
                        if jc >= 2:
                            tensor.wait_ge(s_oc, jc - 1)
                        ci = c0 // 512
                        last = None
                        for k in range(DIM // 128):
                            wait_ev(prev_base + k * ch + ci + 1)
                            last = tensor.matmul(
                                p5[:csz, jc % 2, :OUT_C],
                                hb[:, 1, k, c0 : c0 + csz],
                                w5sb[:, k, :],
                                start=(k == 0),
                                stop=(k == DIM // 128 - 1),
                            )
                        last.then_inc(s_p5, 1)
                        jc += 1

        @block.scalar
        def _(scalar):
            scalar.wait_ge(s_cw, 16)
            mt = 0
            jc = 0
            for p in range(passes):
                for b in range(NBLK):
                    for li, m, ci, off, cs in TILES[b]:
                        t = mt
                        scalar.wait_ge(s_pm, t + 1)
                        scalar.activation(
                            ext[:, t % EXTD, :cs],
                            pz[:, t % NPS, :cs],
                            AF.Exp,
                            bias=bsb[:, li, m : m + 1],
                            scale=1.0,
                        ).then_inc(s_ex, 1)
                        mt += 1
                    for c0, csz in L5_CHUNKS[b]:
                        scalar.wait_ge(s_p5, jc + 1)
                        if jc >= 4:
                            scalar.wait_ge(s_ods[jc % 4], 16 * ((jc - 4) // 4 + 1))
                        scalar.copy(
                            osb[:csz, jc % 4, :], p5[:csz, jc % 2, :OUT_C]
                        ).then_inc(s_oc, 1)
                        jc += 1

        @block.vector
        def _(vector):
            mt = 0
            for p in range(passes):
                for b in range(NBLK):
                    for li, m, ci, off, cs in TILES[b]:
                        t = mt
                        ho = li % 2
                        vector.wait_ge(s_ex, t + 1)
                        vector.tensor_scalar(
                            tmn[:, t % TMND, :cs],
                            ext[:, t % EXTD, :cs],
                            -1.0,
                            0.0,
                            ALU.add,
                            ALU.min,
                        )
                        vector.scalar_tensor_tensor(
                            hb[:, ho, m, off : off + cs],
                            pz[:, t % NPS, :cs],
                            bsb[:, li, m : m + 1],
                            tmn[:, t % TMND, :cs],
                            ALU.add,
                            ALU.max,
                        ).then_inc(s_ev, 1)
                        mt += 1

    nc.compile()
    return nc


def _prep_weights(inputs):
    bf16 = ml_dtypes.bfloat16
    wmaps = {}
    bmin = np.zeros((128, 5, 16), np.float32)
    for l, (ci, co) in enumerate(LAYER_DIMS, start=1):
        W = np.asarray(inputs[f"Wg{l}"], np.float32) + np.asarray(
            inputs[f"Wl{l}"], np.float32
        )
        b = np.asarray(inputs[f"bg{l}"], np.float32) + np.asarray(
            inputs[f"bl{l}"], np.float32
        )
        kc = ci // 128
        if l == 1:
            # [128 part, 16 m, 4 k, 128]
            wmaps["w1"] = np.ascontiguousarray(
                W.reshape(4, 128, 16, 128).transpose(1, 2, 0, 3)
            ).astype(bf16)
            bmin[:, 0, :] = b.reshape(16, 128).T
        elif l < 5:
            wmaps[f"w{l}"] = np.ascontiguousarray(
                W.reshape(kc, 128, co // 128, 128).transpose(2, 1, 0, 3)
            ).astype(bf16)
            bmin[:, l - 1, :] = b.reshape(co // 128, 128).T
        else:
            wmaps["w5"] = np.ascontiguousarray(
                W.reshape(ci // 128, 128, co).transpose(1, 0, 2)
            ).astype(bf16)
            wmaps["_b5"] = b
    wmaps["bmin"] = bmin
    return wmaps


def make_in_maps(inputs):
    x = np.asarray(inputs["x"], np.float32)
    assert x.shape == (N, IN_C)
    wmaps = _prep_weights(inputs)
    b5 = wmaps.pop("_b5")
    in_maps = []
    for c in range(NCORES):
        xs = x[c * NPC : (c + 1) * NPC]
        xt = np.ascontiguousarray(
            xs.T.reshape(IN_C // 128, 128, NPC).transpose(1, 0, 2)
        ).astype(ml_dtypes.bfloat16)
        m = {"x": xt}
        m.update(wmaps)
        in_maps.append(m)
    return in_maps, b5


def kernel(**inputs) -> np.ndarray:
    from concourse.bass_utils import run_bass_kernel_spmd

    in_maps, b5 = make_in_maps(inputs)

    if "nc" not in _cache:
        _cache["nc"] = _build()
    nc = _cache["nc"]

    res = run_bass_kernel_spmd(nc, in_maps, core_ids=list(range(NCORES)))
    out = np.concatenate([res.results[c]["out"] for c in range(NCORES)], axis=0)
    if np.any(b5):
        out = out + b5[None, :]
    return np.ascontiguousarray(out.astype(np.float32))
